# revision 19
# baseline (speedup 1.0000x reference)
"""Channel-attention (CAM) kernel for Trainium2, 8 NeuronCores.

Reference computation (per batch b):
    A   = x[b].reshape(L, C)            # L = 48^3 = 110592, C = 256
    G   = A^T A                          # [C, C] Gram matrix
    S   = softmax(G, axis=-1)
    out = gamma * (A @ S) + x[b]

Sharding: L-parallel across the 8 cores (each core owns L/8 rows of both
batches).  Each core computes a partial Gram over its shard; a per-batch
bf16 AllReduce completes the [C, C] Grams; every core redundantly
computes softmax (tiny), scales it by gamma (zero gamma makes the second
matmul exactly zero, so the output is bit-exact in x), then computes its
shard of A @ (gamma*S) + x with an fp32 x residual.

Layout: [128, RPP, C] "supertiles" — partition p holds RPP consecutive
DRAM rows, giving RPP-KB DMA packets.  The Gram contraction is invariant
to the row permutation, and the transpose / second matmul / residual add
/ store all use the same permuted order consistently.

Memory strategy: batch 0's fp32 supertiles stay resident in SBUF after
phase 1 (108 KB/partition), so phase 2 re-reads only batch 1 from HBM.
Phase 1 is purely the Gram (DMA-bound, PE half idle); all transposes
(PE identity-matmuls; both C-halves of a subtile share one PSUM tile so
a single scalar-engine copy drains them) happen in phase 2, where the
batch-0 transpose work — which does not depend on the AllReduce — fills
the latency gap between phase-1 completion and softmax availability.

Engine-queue discipline (FIFO queues make emission order matter):
  sync    = all x loads + Gram staging DMA
  scalar  = A^T copies, softmax (incl. its DMA), output stores
  vector  = bf16 converts, Gram staging copies, softmax vector ops, adds
  gpsimd  = gamma broadcast + the two AllReduces (blocking waits)
softmax(b) ops are emitted at points where their AllReduce wait cannot
head-of-line-block work that is already runnable.
"""

import numpy as np
from contextlib import ExitStack

import concourse.bass as bass
import concourse.tile as tile
from concourse.bass import _add_dep_helper
from concourse import bacc, mybir
from concourse.bass import ts
from concourse.bass_utils import run_bass_kernel_spmd
from concourse.masks import make_identity

F32 = mybir.dt.float32
BF16 = mybir.dt.bfloat16
AF = mybir.ActivationFunctionType

N_CORES = 8
B = 2
L = 48 * 48 * 48          # 110592
C = 256
L_SH = L // N_CORES       # 13824 rows per core per batch
ROWS = B * L_SH           # 27648 rows per core
P = 128
RPP = 6                   # rows per partition per supertile
HPP = RPP // 2            # rows per partition per half-supertile
SROWS = P * RPP           # 768 rows per supertile
SPB = L_SH // SROWS       # 18 supertiles per batch
S_TOT = B * SPB           # 36 supertiles per core
GAPN = 8                  # transpose-ahead depth (supertiles)

def _resident(s):
    # alternate resident/streamed supertiles within each batch so DMA and
    # vector load stay balanced through both phase-2 legs
    return s % 2 == 0


_CACHE: dict = {}


def _build():
    nc = bacc.Bacc(
        "TRN2", target_bir_lowering=False, debug=False, num_devices=N_CORES
    )
    x_dram = nc.dram_tensor("x", [ROWS, C], F32, kind="ExternalInput")
    g_dram = nc.dram_tensor("gamma", [1, 1], F32, kind="ExternalInput")
    o_dram = nc.dram_tensor("out", [ROWS, C], F32, kind="ExternalOutput")
    cc_in = [
        nc.dram_tensor(f"cc_in{b}", [2 * P, C], BF16, kind="Internal")
        for b in range(B)
    ]
    cc_out = [
        nc.dram_tensor(f"cc_out{b}", [2 * P, C], BF16, kind="Internal")
        for b in range(B)
    ]
    X, GAM, OUT = x_dram.ap(), g_dram.ap(), o_dram.ap()

    def x_super(s):
        return X[ts(s, SROWS), :].rearrange("(p j) c -> p j c", j=RPP)

    def o_super(s):
        return OUT[ts(s, SROWS), :].rearrange("(p j) c -> p j c", j=RPP)

    with tile.TileContext(nc) as tc, ExitStack() as octx:
        constp = octx.enter_context(tc.tile_pool(name="const", bufs=1))
        ident = constp.tile([P, P], BF16, name="ident", tag="ident")
        make_identity(nc, ident[:])
        gam_sb = constp.tile([1, 1], F32, name="gam_sb", tag="gam_sb")
        nc.sync.dma_start(gam_sb[:], GAM[:, :])
        gam_bc = constp.tile([P, 1], F32, name="gam_bc", tag="gam_bc")
        nc.gpsimd.partition_broadcast(gam_bc[:], gam_sb[:])
        s_bf = [
            constp.tile([P, C], BF16, name=f"sbf{i}", tag=f"sbf{i}")
            for i in range(4)
        ]

        # batch-0 fp32 supertiles stay resident from phase 1 through the end
        xres_pool = octx.enter_context(tc.tile_pool(name="xres", bufs=SPB))
        xres: dict = {}

        smst = octx.enter_context(ExitStack())
        sp = smst.enter_context(tc.tile_pool(name="smx", bufs=2))

        # ---------------- phase 1: partial Grams ----------------
        p1 = octx.enter_context(ExitStack())
        xp = p1.enter_context(tc.tile_pool(name="p1x", bufs=6))
        bp = p1.enter_context(tc.tile_pool(name="p1b", bufs=3))
        gp = p1.enter_context(tc.tile_pool(name="p1g", bufs=4))
        psg = p1.enter_context(tc.tile_pool(name="psg", bufs=1, space="PSUM"))
        g_ps = [
            psg.tile([P, C], F32, name=f"gps{i}", tag=f"gps{i}")
            for i in range(4)
        ]

        def phase1_batch(b):
            for si in range(SPB):
                s = b * SPB + si
                if _resident(s):
                    xt = xres_pool.tile([P, RPP, C], F32, name="xr", tag="xr")
                    xres[s] = xt
                else:
                    xt = xp.tile([P, RPP, C], F32, name="x1", tag="x1")
                nc.sync.dma_start(xt[:], x_super(s))
                xb = bp.tile([P, RPP, C], BF16, name="xb1", tag="xb1")
                nc.vector.tensor_copy(xb[:], xt[:])
                for j in range(RPP):
                    first = si == 0 and j == 0
                    last = si == SPB - 1 and j == RPP - 1
                    nc.tensor.matmul(
                        g_ps[2 * b][:], xb[:, j, 0:P], xb[:, j, :],
                        start=first, stop=last,
                    )
                    nc.tensor.matmul(
                        g_ps[2 * b + 1][:], xb[:, j, P:C], xb[:, j, :],
                        start=first, stop=last,
                    )

        def stage_and_ar(b):
            for m in range(2):
                gsb = gp.tile([P, C], BF16, name="gsb", tag="gsb")
                nc.vector.tensor_copy(gsb[:], g_ps[2 * b + m][:])
                nc.sync.dma_start(cc_in[b].ap()[ts(m, P), :], gsb[:])
            nc.gpsimd.collective_compute(
                "AllReduce",
                mybir.AluOpType.add,
                replica_groups=[list(range(N_CORES))],
                ins=[cc_in[b].ap()[:, :]],
                outs=[cc_out[b].ap()[:, :]],
            )

        pending = {"v": [], "a": [], "p": []}

        def order_after(inst, key, why):
            for dep_inst in pending[key]:
                _add_dep_helper(inst.ins, dep_inst.ins, sync=False, reason=why)
            pending[key] = []

        def softmax(b):
            for m in range(2):
                i = 2 * b + m
                gf = sp.tile([P, C], BF16, name="gf", tag="gf")
                gfd = nc.scalar.dma_start(gf[:], cc_out[b].ap()[ts(m, P), :])
                if m == 0:
                    order_after(gfd, "a", f"softmax{b} after pre-AR scalar work")
                nmx = sp.tile([P, 1], F32, name="nmx", tag="nmx")
                nmxi = nc.vector.tensor_reduce(
                    nmx[:],
                    gf[:],
                    axis=mybir.AxisListType.X,
                    op=mybir.AluOpType.max,
                    negate=True,
                )
                if m == 0:
                    order_after(nmxi, "v", f"softmax{b} after pre-AR vector work")
                ex = sp.tile([P, C], F32, name="ex", tag="ex")
                ssum = sp.tile([P, 1], F32, name="ssum", tag="ssum")
                nc.scalar.activation(
                    ex[:], gf[:], AF.Exp, bias=nmx[:], scale=1.0, accum_out=ssum[:]
                )
                inv = sp.tile([P, 1], F32, name="inv", tag="inv")
                nc.vector.reciprocal(inv[:], ssum[:])
                sc = sp.tile([P, 1], F32, name="sc", tag="sc")
                nc.vector.tensor_mul(sc[:], inv[:], gam_bc[:])
                nc.scalar.activation(s_bf[i][:], ex[:], AF.Copy, scale=sc[:])

        phase1_batch(0)
        stage_and_ar(0)
        phase1_batch(1)
        stage_and_ar(1)
        p1.close()

        # ---------------- phase 2: out = A @ s_bf + x ----------------
        with ExitStack() as p2:
            xp2 = p2.enter_context(tc.tile_pool(name="p2x", bufs=5))
            bp2 = p2.enter_context(tc.tile_pool(name="p2b", bufs=3))
            ap2 = p2.enter_context(tc.tile_pool(name="p2a", bufs=GAPN * RPP))
            op2 = p2.enter_context(tc.tile_pool(name="p2o", bufs=4))
            pst = p2.enter_context(tc.tile_pool(name="pst", bufs=2, space="PSUM"))
            psy = p2.enter_context(tc.tile_pool(name="psy", bufs=3, space="PSUM"))

            x2: dict = {}
            ats: dict = {}

            def twork(s):
                # transpose work for supertile s; no AllReduce dependency
                if _resident(s):
                    xt = xres[s]
                else:
                    xt = xp2.tile([P, RPP, C], F32, name="x2", tag="x2")
                    nc.sync.dma_start(xt[:], x_super(s))
                    x2[s] = xt
                xb = bp2.tile([P, RPP, C], BF16, name="xb2", tag="xb2")
                cv = nc.gpsimd.tensor_copy(xb[:], xt[:])
                pending["g"] = pending.get("g", [])
                for j in range(RPP):
                    tp = pst.tile([P, 2, P], F32, name="tp", tag="tp")
                    t0 = nc.tensor.matmul(
                        tp[:, 0, :], xb[:, j, 0:P], ident[:],
                        start=True, stop=False,
                    )
                    t1 = nc.tensor.matmul(
                        tp[:, 1, :], xb[:, j, P:C], ident[:],
                        start=False, stop=True,
                    )
                    pending["p"] += [t0, t1]
                    at = ap2.tile([P, 2, P], BF16, name="at", tag="at")
                    cp = nc.scalar.activation(at[:], tp[:], AF.Copy)
                    pending["a"].append(cp)
                    ats[(s, j)] = at

            def ywork(s, gate_pe=False):
                b = s // SPB
                xt = xres[s] if _resident(s) else x2.pop(s)
                ot = op2.tile([P, RPP, C], F32, name="ot", tag="ot")
                for jh in range(2):
                    y = psy.tile([P, HPP, C], F32, name="y", tag="y")
                    for jj in range(HPP):
                        j = jh * HPP + jj
                        a = ats.pop((s, j))
                        ym = nc.tensor.matmul(
                            y[:, jj, :], a[:, 0, :], s_bf[2 * b][:],
                            start=True, stop=False,
                        )
                        if gate_pe:
                            order_after(ym, "p", "first Y-MM after pending transposes")
                            gate_pe = False
                        nc.tensor.matmul(
                            y[:, jj, :], a[:, 1, :], s_bf[2 * b + 1][:],
                            start=False, stop=True,
                        )
                    ad = nc.vector.tensor_add(
                        ot[:, ts(jh, HPP), :], y[:], xt[:, ts(jh, HPP), :]
                    )
                    pending["v"].append(ad)
                steng = nc.scalar if s < 8 else nc.gpsimd
                st = steng.dma_start(o_super(s), ot[:])
                if s < 8:
                    pending["a"].append(st)

            for s in range(GAPN):
                twork(s)
            softmax(0)
            for s in range(SPB):
                ywork(s, gate_pe=(s == 0))
                if s + GAPN < S_TOT:
                    twork(s + GAPN)
            softmax(1)
            for s in range(SPB, S_TOT):
                ywork(s, gate_pe=(s == SPB))
                if s + GAPN < S_TOT:
                    twork(s + GAPN)
        smst.close()

    nc.compile()
    return nc


def _get_nc():
    if "nc" not in _CACHE:
        _CACHE["nc"] = _build()
    return _CACHE["nc"]


def kernel(x: np.ndarray, gamma: np.ndarray, **_kw) -> np.ndarray:
    nc = _get_nc()
    x = np.asarray(x, dtype=np.float32)
    orig_shape = x.shape
    x3 = x.reshape(B, L, C)
    gam = np.asarray(gamma, dtype=np.float32).reshape(1, 1)
    in_maps = []
    for k in range(N_CORES):
        shard = np.ascontiguousarray(
            x3[:, k * L_SH : (k + 1) * L_SH, :]
        ).reshape(ROWS, C)
        in_maps.append({"x": shard, "gamma": gam})
    res = run_bass_kernel_spmd(nc, in_maps, core_ids=list(range(N_CORES)))
    out = np.empty((B, L, C), dtype=np.float32)
    for k in range(N_CORES):
        out[:, k * L_SH : (k + 1) * L_SH, :] = res.results[k]["out"].reshape(
            B, L_SH, C
        )
    return out.reshape(orig_shape)


# revision 20
# speedup vs baseline: 1.6491x; 1.6491x over previous
"""Channel-attention (CAM) kernel for Trainium2, 8 NeuronCores.

Reference computation (per batch b):
    A   = x[b].reshape(L, C)            # L = 48^3 = 110592, C = 256
    G   = A^T A                          # [C, C] Gram matrix
    S   = softmax(G, axis=-1)
    out = gamma * (A @ S) + x[b]

Sharding: L-parallel across the 8 cores (each core owns L/8 rows of both
batches).  Each core computes a partial Gram over its shard; a per-batch
bf16 AllReduce completes the [C, C] Grams; every core redundantly
computes softmax (tiny), scales it by gamma (zero gamma makes the second
matmul exactly zero, so the output is bit-exact in x), then computes its
shard of A @ (gamma*S) + x with an fp32 x residual.

Layout: [128, RPP, C] "supertiles" — partition p holds RPP consecutive
DRAM rows, giving RPP-KB DMA packets.  The Gram contraction is invariant
to the row permutation, and the transpose / second matmul / residual add
/ store all use the same permuted order consistently.

Memory strategy: batch 0's fp32 supertiles stay resident in SBUF after
phase 1 (108 KB/partition), so phase 2 re-reads only batch 1 from HBM.
Phase 1 is purely the Gram (DMA-bound, PE half idle); all transposes
(PE identity-matmuls; both C-halves of a subtile share one PSUM tile so
a single scalar-engine copy drains them) happen in phase 2, where the
batch-0 transpose work — which does not depend on the AllReduce — fills
the latency gap between phase-1 completion and softmax availability.

Engine-queue discipline (FIFO queues make emission order matter):
  sync    = all x loads + Gram staging DMA
  scalar  = A^T copies, softmax (incl. its DMA), output stores
  vector  = bf16 converts, Gram staging copies, softmax vector ops, adds
  gpsimd  = gamma broadcast + the two AllReduces (blocking waits)
softmax(b) ops are emitted at points where their AllReduce wait cannot
head-of-line-block work that is already runnable.
"""

import numpy as np
from contextlib import ExitStack

import concourse.bass as bass
import concourse.tile as tile
from concourse.bass import _add_dep_helper
from concourse import bacc, mybir
from concourse.bass import ts
from concourse.bass_utils import run_bass_kernel_spmd
from concourse.masks import make_identity

F32 = mybir.dt.float32
BF16 = mybir.dt.bfloat16
AF = mybir.ActivationFunctionType

N_CORES = 8
B = 2
L = 48 * 48 * 48          # 110592
C = 256
L_SH = L // N_CORES       # 13824 rows per core per batch
ROWS = B * L_SH           # 27648 rows per core
P = 128
RPP = 6                   # rows per partition per supertile
HPP = RPP // 2            # rows per partition per half-supertile
SROWS = P * RPP           # 768 rows per supertile
SPB = L_SH // SROWS       # 18 supertiles per batch
S_TOT = B * SPB           # 36 supertiles per core
GAPN = 8                  # transpose-ahead depth (supertiles)

def _resident(s):
    # alternate resident/streamed supertiles within each batch so DMA and
    # vector load stay balanced through both phase-2 legs
    return s % 2 == 0


_CACHE: dict = {}


def _build():
    nc = bacc.Bacc(
        "TRN2", target_bir_lowering=False, debug=False, num_devices=N_CORES
    )
    x_dram = nc.dram_tensor("x", [ROWS, C], F32, kind="ExternalInput")
    g_dram = nc.dram_tensor("gamma", [1, 1], F32, kind="ExternalInput")
    o_dram = nc.dram_tensor("out", [ROWS, C], F32, kind="ExternalOutput")
    cc_in = [
        nc.dram_tensor(f"cc_in{b}", [2 * P, C], BF16, kind="Internal")
        for b in range(B)
    ]
    cc_out = [
        nc.dram_tensor(f"cc_out{b}", [2 * P, C], BF16, kind="Internal")
        for b in range(B)
    ]
    X, GAM, OUT = x_dram.ap(), g_dram.ap(), o_dram.ap()

    def x_super(s):
        return X[ts(s, SROWS), :].rearrange("(p j) c -> p j c", j=RPP)

    def o_super(s):
        return OUT[ts(s, SROWS), :].rearrange("(p j) c -> p j c", j=RPP)

    with tile.TileContext(nc) as tc, ExitStack() as octx:
        constp = octx.enter_context(tc.tile_pool(name="const", bufs=1))
        ident = constp.tile([P, P], BF16, name="ident", tag="ident")
        make_identity(nc, ident[:])
        gam_sb = constp.tile([1, 1], F32, name="gam_sb", tag="gam_sb")
        nc.sync.dma_start(gam_sb[:], GAM[:, :])
        gam_bc = constp.tile([P, 1], F32, name="gam_bc", tag="gam_bc")
        nc.gpsimd.partition_broadcast(gam_bc[:], gam_sb[:])
        s_bf = [
            constp.tile([P, C], BF16, name=f"sbf{i}", tag=f"sbf{i}")
            for i in range(4)
        ]

        # batch-0 fp32 supertiles stay resident from phase 1 through the end
        xres_pool = octx.enter_context(tc.tile_pool(name="xres", bufs=SPB))
        xres: dict = {}

        smst = octx.enter_context(ExitStack())
        sp = smst.enter_context(tc.tile_pool(name="smx", bufs=2))

        # ---------------- phase 1: partial Grams ----------------
        p1 = octx.enter_context(ExitStack())
        xp = p1.enter_context(tc.tile_pool(name="p1x", bufs=6))
        bp = p1.enter_context(tc.tile_pool(name="p1b", bufs=3))
        gp = p1.enter_context(tc.tile_pool(name="p1g", bufs=4))
        psg = p1.enter_context(tc.tile_pool(name="psg", bufs=1, space="PSUM"))
        g_ps = [
            psg.tile([P, C], F32, name=f"gps{i}", tag=f"gps{i}")
            for i in range(4)
        ]

        def phase1_batch(b):
            for si in range(SPB):
                s = b * SPB + si
                if _resident(s):
                    xt = xres_pool.tile([P, RPP, C], F32, name="xr", tag="xr")
                    xres[s] = xt
                else:
                    xt = xp.tile([P, RPP, C], F32, name="x1", tag="x1")
                nc.sync.dma_start(xt[:], x_super(s))
                xb = bp.tile([P, RPP, C], BF16, name="xb1", tag="xb1")
                nc.vector.tensor_copy(xb[:], xt[:])
                for j in range(RPP):
                    first = si == 0 and j == 0
                    last = si == SPB - 1 and j == RPP - 1
                    nc.tensor.matmul(
                        g_ps[2 * b][:], xb[:, j, 0:P], xb[:, j, :],
                        start=first, stop=last,
                    )
                    nc.tensor.matmul(
                        g_ps[2 * b + 1][:], xb[:, j, P:C], xb[:, j, :],
                        start=first, stop=last,
                    )

        def stage_and_ar(b):
            for m in range(2):
                gsb = gp.tile([P, C], BF16, name="gsb", tag="gsb")
                nc.vector.tensor_copy(gsb[:], g_ps[2 * b + m][:])
                nc.sync.dma_start(cc_in[b].ap()[ts(m, P), :], gsb[:])
            nc.gpsimd.collective_compute(
                "AllReduce",
                mybir.AluOpType.add,
                replica_groups=[list(range(N_CORES))],
                ins=[cc_in[b].ap()[:, :]],
                outs=[cc_out[b].ap()[:, :]],
            )

        pending = {"v": [], "a": [], "p": []}

        def order_after(inst, key, why):
            for dep_inst in pending[key]:
                _add_dep_helper(inst.ins, dep_inst.ins, sync=False, reason=why)
            pending[key] = []

        def softmax(b):
            for m in range(2):
                i = 2 * b + m
                gf = sp.tile([P, C], BF16, name="gf", tag="gf")
                gfd = nc.scalar.dma_start(gf[:], cc_out[b].ap()[ts(m, P), :])
                if m == 0:
                    order_after(gfd, "a", f"softmax{b} after pre-AR scalar work")
                nmx = sp.tile([P, 1], F32, name="nmx", tag="nmx")
                nmxi = nc.vector.tensor_reduce(
                    nmx[:],
                    gf[:],
                    axis=mybir.AxisListType.X,
                    op=mybir.AluOpType.max,
                    negate=True,
                )
                if m == 0:
                    order_after(nmxi, "v", f"softmax{b} after pre-AR vector work")
                ex = sp.tile([P, C], F32, name="ex", tag="ex")
                ssum = sp.tile([P, 1], F32, name="ssum", tag="ssum")
                nc.scalar.activation(
                    ex[:], gf[:], AF.Exp, bias=nmx[:], scale=1.0, accum_out=ssum[:]
                )
                inv = sp.tile([P, 1], F32, name="inv", tag="inv")
                nc.vector.reciprocal(inv[:], ssum[:])
                sc = sp.tile([P, 1], F32, name="sc", tag="sc")
                nc.vector.tensor_mul(sc[:], inv[:], gam_bc[:])
                nc.scalar.activation(s_bf[i][:], ex[:], AF.Copy, scale=sc[:])

        phase1_batch(0)
        stage_and_ar(0)
        phase1_batch(1)
        stage_and_ar(1)
        p1.close()

        # ---------------- phase 2: out = A @ s_bf + x ----------------
        with ExitStack() as p2:
            xp2 = p2.enter_context(tc.tile_pool(name="p2x", bufs=5))
            bp2 = p2.enter_context(tc.tile_pool(name="p2b", bufs=3))
            ap2 = p2.enter_context(tc.tile_pool(name="p2a", bufs=GAPN * RPP))
            op2 = p2.enter_context(tc.tile_pool(name="p2o", bufs=4))
            pst = p2.enter_context(tc.tile_pool(name="pst", bufs=2, space="PSUM"))
            psy = p2.enter_context(tc.tile_pool(name="psy", bufs=3, space="PSUM"))

            x2: dict = {}
            ats: dict = {}

            def twork(s):
                # transpose work for supertile s; no AllReduce dependency
                if _resident(s):
                    xt = xres[s]
                else:
                    xt = xp2.tile([P, RPP, C], F32, name="x2", tag="x2")
                    nc.sync.dma_start(xt[:], x_super(s))
                    x2[s] = xt
                xb = bp2.tile([P, RPP, C], BF16, name="xb2", tag="xb2")
                cv = nc.vector.tensor_copy(xb[:], xt[:])
                pending["v"].append(cv)
                for j in range(RPP):
                    tp = pst.tile([P, 2, P], F32, name="tp", tag="tp")
                    t0 = nc.tensor.matmul(
                        tp[:, 0, :], xb[:, j, 0:P], ident[:],
                        start=True, stop=False,
                    )
                    t1 = nc.tensor.matmul(
                        tp[:, 1, :], xb[:, j, P:C], ident[:],
                        start=False, stop=True,
                    )
                    pending["p"] += [t0, t1]
                    at = ap2.tile([P, 2, P], BF16, name="at", tag="at")
                    cp = nc.scalar.activation(at[:], tp[:], AF.Copy)
                    pending["a"].append(cp)
                    ats[(s, j)] = at

            def ywork(s, gate_pe=False):
                b = s // SPB
                xt = xres[s] if _resident(s) else x2.pop(s)
                ot = op2.tile([P, RPP, C], F32, name="ot", tag="ot")
                for jh in range(2):
                    y = psy.tile([P, HPP, C], F32, name="y", tag="y")
                    for jj in range(HPP):
                        j = jh * HPP + jj
                        a = ats.pop((s, j))
                        ym = nc.tensor.matmul(
                            y[:, jj, :], a[:, 0, :], s_bf[2 * b][:],
                            start=True, stop=False,
                        )
                        if gate_pe:
                            order_after(ym, "p", "first Y-MM after pending transposes")
                            gate_pe = False
                        nc.tensor.matmul(
                            y[:, jj, :], a[:, 1, :], s_bf[2 * b + 1][:],
                            start=False, stop=True,
                        )
                    ad = nc.vector.tensor_add(
                        ot[:, ts(jh, HPP), :], y[:], xt[:, ts(jh, HPP), :]
                    )
                    pending["v"].append(ad)
                steng = nc.scalar if s < 8 else nc.gpsimd
                st = steng.dma_start(o_super(s), ot[:])
                if s < 8:
                    pending["a"].append(st)

            for s in range(GAPN):
                twork(s)
            softmax(0)
            for s in range(SPB):
                ywork(s, gate_pe=(s == 0))
                if s + GAPN < S_TOT:
                    twork(s + GAPN)
            softmax(1)
            for s in range(SPB, S_TOT):
                ywork(s, gate_pe=(s == SPB))
                if s + GAPN < S_TOT:
                    twork(s + GAPN)
        smst.close()

    nc.compile()
    return nc


def _get_nc():
    if "nc" not in _CACHE:
        _CACHE["nc"] = _build()
    return _CACHE["nc"]


def kernel(x: np.ndarray, gamma: np.ndarray, **_kw) -> np.ndarray:
    nc = _get_nc()
    x = np.asarray(x, dtype=np.float32)
    orig_shape = x.shape
    x3 = x.reshape(B, L, C)
    gam = np.asarray(gamma, dtype=np.float32).reshape(1, 1)
    in_maps = []
    for k in range(N_CORES):
        shard = np.ascontiguousarray(
            x3[:, k * L_SH : (k + 1) * L_SH, :]
        ).reshape(ROWS, C)
        in_maps.append({"x": shard, "gamma": gam})
    res = run_bass_kernel_spmd(nc, in_maps, core_ids=list(range(N_CORES)))
    out = np.empty((B, L, C), dtype=np.float32)
    for k in range(N_CORES):
        out[:, k * L_SH : (k + 1) * L_SH, :] = res.results[k]["out"].reshape(
            B, L_SH, C
        )
    return out.reshape(orig_shape)


# revision 21
# speedup vs baseline: 1.7622x; 1.0686x over previous
"""Channel-attention (CAM) kernel for Trainium2, 8 NeuronCores.

Reference computation (per batch b):
    A   = x[b].reshape(L, C)            # L = 48^3 = 110592, C = 256
    G   = A^T A                          # [C, C] Gram matrix
    S   = softmax(G, axis=-1)
    out = gamma * (A @ S) + x[b]

Sharding: L-parallel across the 8 cores (each core owns L/8 rows of both
batches).  Each core computes a partial Gram over its shard; a per-batch
bf16 AllReduce completes the [C, C] Grams; every core redundantly
computes softmax (tiny), scales it by gamma (zero gamma makes the second
matmul exactly zero, so the output is bit-exact in x), then computes its
shard of A @ (gamma*S) + x with an fp32 x residual.

Layout: [128, RPP, C] "supertiles" — partition p holds RPP consecutive
DRAM rows, giving RPP-KB DMA packets.  The Gram contraction is invariant
to the row permutation, and the transpose / second matmul / residual add
/ store all use the same permuted order consistently.

Memory strategy: batch 0's fp32 supertiles stay resident in SBUF after
phase 1 (108 KB/partition), so phase 2 re-reads only batch 1 from HBM.
Phase 1 is purely the Gram (DMA-bound, PE half idle); all transposes
(PE identity-matmuls; both C-halves of a subtile share one PSUM tile so
a single scalar-engine copy drains them) happen in phase 2, where the
batch-0 transpose work — which does not depend on the AllReduce — fills
the latency gap between phase-1 completion and softmax availability.

Engine-queue discipline (FIFO queues make emission order matter):
  sync    = all x loads + Gram staging DMA
  scalar  = A^T copies, softmax (incl. its DMA), output stores
  vector  = bf16 converts, Gram staging copies, softmax vector ops, adds
  gpsimd  = gamma broadcast + the two AllReduces (blocking waits)
softmax(b) ops are emitted at points where their AllReduce wait cannot
head-of-line-block work that is already runnable.
"""

import numpy as np
from contextlib import ExitStack

import concourse.bass as bass
import concourse.tile as tile
from concourse.bass import _add_dep_helper
from concourse import bacc, mybir
from concourse.bass import ts
from concourse.bass_utils import run_bass_kernel_spmd
from concourse.masks import make_identity

F32 = mybir.dt.float32
BF16 = mybir.dt.bfloat16
AF = mybir.ActivationFunctionType

N_CORES = 8
B = 2
L = 48 * 48 * 48          # 110592
C = 256
L_SH = L // N_CORES       # 13824 rows per core per batch
ROWS = B * L_SH           # 27648 rows per core
P = 128
RPP = 6                   # rows per partition per supertile
HPP = RPP // 2            # rows per partition per half-supertile
SROWS = P * RPP           # 768 rows per supertile
SPB = L_SH // SROWS       # 18 supertiles per batch
S_TOT = B * SPB           # 36 supertiles per core
GAPN = 8                  # transpose-ahead depth (supertiles)

def _resident(s):
    # alternate resident/streamed supertiles within each batch so DMA and
    # vector load stay balanced through both phase-2 legs
    return s % 2 == 0


_CACHE: dict = {}


def _build():
    nc = bacc.Bacc(
        "TRN2", target_bir_lowering=False, debug=False, num_devices=N_CORES
    )
    x_dram = nc.dram_tensor("x", [ROWS, C], F32, kind="ExternalInput")
    g_dram = nc.dram_tensor("gamma", [1, 1], F32, kind="ExternalInput")
    o_dram = nc.dram_tensor("out", [ROWS, C], F32, kind="ExternalOutput")
    cc_in = [
        nc.dram_tensor(f"cc_in{b}", [2 * P, C], BF16, kind="Internal")
        for b in range(B)
    ]
    cc_out = [
        nc.dram_tensor(f"cc_out{b}", [2 * P, C], BF16, kind="Internal")
        for b in range(B)
    ]
    X, GAM, OUT = x_dram.ap(), g_dram.ap(), o_dram.ap()

    def x_super(s):
        return X[ts(s, SROWS), :].rearrange("(p j) c -> p j c", j=RPP)

    def o_super(s):
        return OUT[ts(s, SROWS), :].rearrange("(p j) c -> p j c", j=RPP)

    with tile.TileContext(nc) as tc, ExitStack() as octx:
        constp = octx.enter_context(tc.tile_pool(name="const", bufs=1))
        ident = constp.tile([P, P], BF16, name="ident", tag="ident")
        make_identity(nc, ident[:])
        gam_sb = constp.tile([1, 1], F32, name="gam_sb", tag="gam_sb")
        nc.sync.dma_start(gam_sb[:], GAM[:, :])
        gam_bc = constp.tile([P, 1], F32, name="gam_bc", tag="gam_bc")
        nc.gpsimd.partition_broadcast(gam_bc[:], gam_sb[:])
        s_bf = [
            constp.tile([P, C], BF16, name=f"sbf{i}", tag=f"sbf{i}")
            for i in range(4)
        ]

        # batch-0 fp32 supertiles stay resident from phase 1 through the end
        xres_pool = octx.enter_context(tc.tile_pool(name="xres", bufs=SPB))
        xres: dict = {}

        smst = octx.enter_context(ExitStack())
        sp = smst.enter_context(tc.tile_pool(name="smx", bufs=2))

        # ---------------- phase 1: partial Grams ----------------
        p1 = octx.enter_context(ExitStack())
        xp = p1.enter_context(tc.tile_pool(name="p1x", bufs=6))
        bp = p1.enter_context(tc.tile_pool(name="p1b", bufs=3))
        gp = p1.enter_context(tc.tile_pool(name="p1g", bufs=4))
        psg = p1.enter_context(tc.tile_pool(name="psg", bufs=1, space="PSUM"))
        g_ps = [
            psg.tile([P, C], F32, name=f"gps{i}", tag=f"gps{i}")
            for i in range(4)
        ]

        def phase1_batch(b):
            for si in range(SPB):
                s = b * SPB + si
                if _resident(s):
                    xt = xres_pool.tile([P, RPP, C], F32, name="xr", tag="xr")
                    xres[s] = xt
                else:
                    xt = xp.tile([P, RPP, C], F32, name="x1", tag="x1")
                nc.sync.dma_start(xt[:], x_super(s))
                xb = bp.tile([P, RPP, C], BF16, name="xb1", tag="xb1")
                nc.vector.tensor_copy(xb[:], xt[:])
                for j in range(RPP):
                    first = si == 0 and j == 0
                    last = si == SPB - 1 and j == RPP - 1
                    nc.tensor.matmul(
                        g_ps[2 * b][:], xb[:, j, 0:P], xb[:, j, :],
                        start=first, stop=last,
                    )
                    nc.tensor.matmul(
                        g_ps[2 * b + 1][:], xb[:, j, P:C], xb[:, j, :],
                        start=first, stop=last,
                    )

        def stage_and_ar(b):
            for m in range(2):
                gsb = gp.tile([P, C], BF16, name="gsb", tag="gsb")
                nc.vector.tensor_copy(gsb[:], g_ps[2 * b + m][:])
                nc.sync.dma_start(cc_in[b].ap()[ts(m, P), :], gsb[:])
            nc.gpsimd.collective_compute(
                "AllReduce",
                mybir.AluOpType.add,
                replica_groups=[list(range(N_CORES))],
                ins=[cc_in[b].ap()[:, :]],
                outs=[cc_out[b].ap()[:, :]],
            )

        pending = {"v": [], "a": [], "p": []}

        def order_after(inst, key, why):
            for dep_inst in pending[key]:
                _add_dep_helper(inst.ins, dep_inst.ins, sync=False, reason=why)
            pending[key] = []

        def softmax(b):
            for m in range(2):
                i = 2 * b + m
                gf = sp.tile([P, C], BF16, name="gf", tag="gf")
                gfd = nc.scalar.dma_start(gf[:], cc_out[b].ap()[ts(m, P), :])
                if m == 0:
                    order_after(gfd, "a", f"softmax{b} after pre-AR scalar work")
                nmx = sp.tile([P, 1], F32, name="nmx", tag="nmx")
                nmxi = nc.vector.tensor_reduce(
                    nmx[:],
                    gf[:],
                    axis=mybir.AxisListType.X,
                    op=mybir.AluOpType.max,
                    negate=True,
                )
                if m == 0:
                    order_after(nmxi, "v", f"softmax{b} after pre-AR vector work")
                ex = sp.tile([P, C], F32, name="ex", tag="ex")
                ssum = sp.tile([P, 1], F32, name="ssum", tag="ssum")
                nc.scalar.activation(
                    ex[:], gf[:], AF.Exp, bias=nmx[:], scale=1.0, accum_out=ssum[:]
                )
                inv = sp.tile([P, 1], F32, name="inv", tag="inv")
                nc.vector.reciprocal(inv[:], ssum[:])
                sc = sp.tile([P, 1], F32, name="sc", tag="sc")
                nc.vector.tensor_mul(sc[:], inv[:], gam_bc[:])
                nc.scalar.activation(s_bf[i][:], ex[:], AF.Copy, scale=sc[:])

        phase1_batch(0)
        stage_and_ar(0)
        phase1_batch(1)
        stage_and_ar(1)
        p1.close()

        # ---------------- phase 2: out = A @ s_bf + x ----------------
        with ExitStack() as p2:
            xp2 = p2.enter_context(tc.tile_pool(name="p2x", bufs=5))
            bp2 = p2.enter_context(tc.tile_pool(name="p2b", bufs=3))
            ap2 = p2.enter_context(tc.tile_pool(name="p2a", bufs=GAPN * RPP))
            op2 = p2.enter_context(tc.tile_pool(name="p2o", bufs=4))
            pst = p2.enter_context(tc.tile_pool(name="pst", bufs=3, space="PSUM"))
            psy = p2.enter_context(tc.tile_pool(name="psy", bufs=2, space="PSUM"))

            x2: dict = {}
            ats: dict = {}

            def twork(s):
                # transpose work for supertile s; no AllReduce dependency
                if _resident(s):
                    xt = xres[s]
                else:
                    xt = xp2.tile([P, RPP, C], F32, name="x2", tag="x2")
                    nc.sync.dma_start(xt[:], x_super(s))
                    x2[s] = xt
                xb = bp2.tile([P, RPP, C], BF16, name="xb2", tag="xb2")
                cv = nc.vector.tensor_copy(xb[:], xt[:])
                pending["v"].append(cv)
                for j in range(RPP):
                    tp = pst.tile([P, 2, P], F32, name="tp", tag="tp")
                    t0 = nc.tensor.matmul(
                        tp[:, 0, :], xb[:, j, 0:P], ident[:],
                        start=True, stop=False,
                    )
                    t1 = nc.tensor.matmul(
                        tp[:, 1, :], xb[:, j, P:C], ident[:],
                        start=False, stop=True,
                    )
                    pending["p"] += [t0, t1]
                    at = ap2.tile([P, 2, P], BF16, name="at", tag="at")
                    cp = nc.scalar.activation(at[:], tp[:], AF.Copy)
                    pending["a"].append(cp)
                    ats[(s, j)] = at

            def ywork(s, gate_pe=False):
                b = s // SPB
                xt = xres[s] if _resident(s) else x2.pop(s)
                ot = op2.tile([P, RPP, C], F32, name="ot", tag="ot")
                for jh in range(2):
                    y = psy.tile([P, HPP, C], F32, name="y", tag="y")
                    for jj in range(HPP):
                        j = jh * HPP + jj
                        a = ats.pop((s, j))
                        ym = nc.tensor.matmul(
                            y[:, jj, :], a[:, 0, :], s_bf[2 * b][:],
                            start=True, stop=False,
                        )
                        if gate_pe:
                            order_after(ym, "p", "first Y-MM after pending transposes")
                            gate_pe = False
                        nc.tensor.matmul(
                            y[:, jj, :], a[:, 1, :], s_bf[2 * b + 1][:],
                            start=False, stop=True,
                        )
                    ad = nc.vector.tensor_add(
                        ot[:, ts(jh, HPP), :], y[:], xt[:, ts(jh, HPP), :]
                    )
                    pending["v"].append(ad)
                steng = nc.scalar if s < 8 else nc.gpsimd
                st = steng.dma_start(o_super(s), ot[:])
                if s < 8:
                    pending["a"].append(st)

            for s in range(GAPN):
                twork(s)
            softmax(0)
            for s in range(SPB):
                ywork(s, gate_pe=(s == 0))
                if s + GAPN < S_TOT:
                    twork(s + GAPN)
            softmax(1)
            for s in range(SPB, S_TOT):
                ywork(s, gate_pe=(s == SPB))
                if s + GAPN < S_TOT:
                    twork(s + GAPN)
        smst.close()

    nc.compile()
    return nc


def _get_nc():
    if "nc" not in _CACHE:
        _CACHE["nc"] = _build()
    return _CACHE["nc"]


def kernel(x: np.ndarray, gamma: np.ndarray, **_kw) -> np.ndarray:
    nc = _get_nc()
    x = np.asarray(x, dtype=np.float32)
    orig_shape = x.shape
    x3 = x.reshape(B, L, C)
    gam = np.asarray(gamma, dtype=np.float32).reshape(1, 1)
    in_maps = []
    for k in range(N_CORES):
        shard = np.ascontiguousarray(
            x3[:, k * L_SH : (k + 1) * L_SH, :]
        ).reshape(ROWS, C)
        in_maps.append({"x": shard, "gamma": gam})
    res = run_bass_kernel_spmd(nc, in_maps, core_ids=list(range(N_CORES)))
    out = np.empty((B, L, C), dtype=np.float32)
    for k in range(N_CORES):
        out[:, k * L_SH : (k + 1) * L_SH, :] = res.results[k]["out"].reshape(
            B, L_SH, C
        )
    return out.reshape(orig_shape)
